# revision 52
# baseline (speedup 1.0000x reference)
"""Multi-head attention with RoPE on 8 Trainium2 NeuronCores (v4).

Problem: x[2,2048,1024] -> MHA(16 heads, hd=64, NeoX RoPE, non-causal) -> out.

Sharding: tensor-parallel over heads. Each core owns 2 heads. All input
layout work (x^T, bf16 casts, rope tables, weight swizzles, per-core
weight column slices) happens host-side in make_in_maps; the device
kernel is pure compute:

  - chunked x^T loads ordered by consumption, with the startup-critical
    pieces on the gpsimd DMA ring (earliest to post) and batch-1's x^T
    reusing batch-0's buffers so its loads carry a WAR semaphore that
    keeps them out of the startup window,
  - rope cos/sin tables load as their unique 32 rows and replicate to 128
    partitions with two SBUF->SBUF copies,
  - q^T,k^T (RoPE'd via a permutation matmul) and v^T projections from the
    pre-transposed x^T, full sequence per core,
  - flash-style attention with *transposed* scores [s_k, s_q]; the softmax
    denominator comes from a fused ones-column in V (constant bias inside
    the exp keeps fp32 range safe),
  - AllToAll staging DMAs are issued per attention chunk, so when the last
    chunk finishes only its own slice remains to stage. Batch 0 goes as
    one piece (peer j owns tokens [j*256,(j+1)*256)); batch 1 ships in
    three pieces (half + quarter + quarter, peer j owning a contiguous
    block of each) so each piece's collective fires as soon as its chunks
    finish and only the last quarter-size piece is exposed at the end,
  - recv DMAs live on the gpsimd queue (which also owns the collective
    triggers, so data deps order naturally and waits stall an idle
    engine); the consumer-side 1/sigma normalize chains + Wo matmuls sit
    in wait-gated tail blocks, filling the last collective's latency,
  - per-512-column output DMAs drain as each Wo column block finishes.

All matmuls run in bf16 (fp32 PSUM accumulation); rel-err tolerance 2e-2.
"""

import sys

sys.path.insert(0, "/opt/trn_rl_repo")

import numpy as np  # noqa: E402
import ml_dtypes  # noqa: E402

import concourse.bass as bass  # noqa: E402
import concourse.mybir as mybir  # noqa: E402
import concourse.tile as tile  # noqa: E402
from concourse.bass_utils import run_bass_kernel_spmd  # noqa: E402


N_CORES = 8
D = 1024
H = 16
HD = 64
HL = H // N_CORES  # local heads per core
DL = HL * HD  # 128 local attn dims
EXP_SCALE = 0.125  # 1/sqrt(hd)
EXP_BIAS = -24.0  # exp(s/8 - 24): cancels in softmax, keeps fp32 range safe
GMAX = 2  # score-psum kt-tiles per exp instruction

F32 = mybir.dt.float32
BF16 = mybir.dt.bfloat16
BF16_NP = ml_dtypes.bfloat16


def _kt_groups(kt):
    groups = []
    k0 = 0
    while k0 < kt:
        g = min(GMAX, kt - k0)
        if (kt - k0) % GMAX == 1 and GMAX > 1:
            g = min(GMAX - 1, kt - k0)
        groups.append((k0, g))
        k0 += g
    return groups


def _perm_matrix():
    """lhsT for the rotate_half matmul: qrot^T = lhsT.T @ q^T."""
    mt = np.zeros((DL, DL), dtype=np.float32)
    for o in (0, HD):
        for r in range(HD // 2):
            mt[o + r, o + r + HD // 2] = -1.0
            mt[o + r + HD // 2, o + r] = 1.0
    return np.ascontiguousarray(mt.T)


def split_excess_waits(nc, max_waits=1):
    """This container's walrus rejects >1 semaphore wait per instruction;
    split excess waits onto NoOp carriers on the same engine."""
    for bb in nc.m.functions[0].blocks:
        insts = bb.instructions
        idx = 0
        while idx < len(insts):
            ins = insts[idx]
            si = ins.sync_info
            if si is not None and si.on_wait and len(si.on_wait) > max_waits:
                ow = list(si.on_wait)
                si.on_wait = ow[-max_waits:]
                extra = ow[:-max_waits]
                k = 0
                while extra:
                    chunk, extra = extra[:max_waits], extra[max_waits:]
                    c = mybir.InstNoOp(name=f"{ins.name}-ws{k}", ins=[], outs=[])
                    c.engine = ins.engine
                    c.sync_info = mybir.SyncInfo(on_wait=chunk, on_update=[])
                    nc.register_instruction(c)
                    insts.insert(idx, c)
                    idx += 1
                    k += 1
            idx += 1


def build_nc(b=2, s=2048, chunk=512, pt_bufs=10):
    kt = s // 128
    nch = s // chunk
    dt8 = D // 128
    shard_b0 = s // N_CORES  # 256 tokens per core, batch 0
    shard_b1 = s // (2 * N_CORES)  # 128 tokens per core per piece, batch 1
    groups = _kt_groups(kt)

    nc = bass.Bass()
    # all layout prep is host-side; everything below is bf16 device-ready
    # x^T chunked: [128, b*nch*dt8, chunk], (batch, chunk) blocks contiguous
    xtp = nc.declare_dram_parameter(
        "xt", [128, b * nch * dt8, chunk], BF16, isOutput=False
    )
    csp = nc.declare_dram_parameter("csn", [32, s], BF16, isOutput=False)
    snp = nc.declare_dram_parameter("snn", [32, s], BF16, isOutput=False)
    wqp = nc.declare_dram_parameter("wq", [128, dt8, DL], BF16, isOutput=False)
    wkp = nc.declare_dram_parameter("wk", [128, dt8, DL], BF16, isOutput=False)
    wvp = nc.declare_dram_parameter("wv", [128, dt8, DL], BF16, isOutput=False)
    wop = nc.declare_dram_parameter("wo", [128, dt8, D], BF16, isOutput=False)
    selp = nc.declare_dram_parameter("sel", [H, N_CORES, 128], BF16, isOutput=False)
    mpp = nc.declare_dram_parameter("mperm", [DL, DL], BF16, isOutput=False)
    idp = nc.declare_dram_parameter("ident", [128, 128], BF16, isOutput=False)
    out = nc.declare_dram_parameter("out", [4 * shard_b1, D], F32, isOutput=True)

    def xt_dram(bi, ch):
        o = (bi * nch + ch) * dt8
        return xtp[:, o : o + dt8, :]

    with tile.TileContext(nc) as tc:
        with (
            tc.tile_pool(name="dram", bufs=1, space="DRAM") as dram,
            tc.tile_pool(name="const", bufs=1) as cpool,
            tc.tile_pool(name="xt", bufs=1) as xtpool,
            tc.tile_pool(name="qkv", bufs=2) as qkvpool,
            tc.tile_pool(name="rope", bufs=2) as ropepool,
            tc.tile_pool(name="pt", bufs=pt_bufs) as ptpool,
            tc.tile_pool(name="att", bufs=2) as attpool,
            tc.tile_pool(name="nrm", bufs=2) as nrmpool,
            tc.tile_pool(name="recv", bufs=1) as rcvpool,
            tc.tile_pool(name="outp", bufs=2) as outpool,
            # PSUM: 8 banks. psA = scores (2 tags x 2 banks; Wo borrows).
            # psB = 2 PV banks. psC = 2 banks for proj / v-transposes / rot /
            # sigma broadcasts.
            tc.tile_pool(name="psA", bufs=1, space="PSUM") as psA,
            tc.tile_pool(name="psB", bufs=2, space="PSUM") as psB,
            tc.tile_pool(name="psC", bufs=2, space="PSUM") as psC,
        ):
            # ---------- loads, critical-path first ----------
            # gpsimd posts DMAs earliest (its preamble is shortest), so the
            # startup-critical loads go there, in consumption order. xt1/wo
            # post later from the sync ring behind a WAR semaphore (below)
            # so their descriptors can't jump ahead of these in the queues.
            wk_sb = cpool.tile([128, dt8, DL], BF16, tag="wk")
            nc.gpsimd.dma_start(wk_sb[:], wkp[:])
            xt0 = [
                xtpool.tile([128, dt8, chunk], BF16, tag=f"xt0c{ch}",
                            name=f"xt0c{ch}")
                for ch in range(nch)
            ]
            nc.gpsimd.dma_start(xt0[0][:], xt_dram(0, 0))
            # rope tables: load the unique 32 rows, replicate to 128
            # partitions with two SBUF->SBUF copies each (saves 0.75MB of
            # contended startup HBM traffic)
            cs128 = cpool.tile([128, s], BF16, tag="cs")
            nc.gpsimd.dma_start(cs128[0:32, :], csp[:])
            sn128 = cpool.tile([128, s], BF16, tag="sn")
            nc.gpsimd.dma_start(sn128[0:32, :], snp[:])
            nc.gpsimd.dma_start(xt0[1][:], xt_dram(0, 1))
            mp_sb = cpool.tile([DL, DL], BF16, tag="mperm")
            nc.gpsimd.dma_start(mp_sb[:], mpp[:])
            wv_sb = cpool.tile([128, dt8, DL], BF16, tag="wv")
            nc.gpsimd.dma_start(wv_sb[:], wvp[:])
            # table broadcasts ride the same gpsimd ring AFTER the critical
            # loads (FIFO): they can't steal queue time from xt0c0
            for tbl in (cs128, sn128):
                nc.gpsimd.dma_start(tbl[32:64, :], tbl[0:32, :])
                nc.gpsimd.dma_start(tbl[64:128, :], tbl[0:64, :])
            id_sb = cpool.tile([128, 128], BF16, tag="ident")
            nc.gpsimd.dma_start(id_sb[:], idp[:])
            nc.gpsimd.dma_start(xt0[2][:], xt_dram(0, 2))
            nc.gpsimd.dma_start(xt0[3][:], xt_dram(0, 3))
            wq_sb = cpool.tile([128, dt8, DL], BF16, tag="wq")
            nc.gpsimd.dma_start(wq_sb[:], wqp[:])
            sel_sb = cpool.tile([H, N_CORES, 128], BF16, tag="sel")
            nc.gpsimd.dma_start(sel_sb[:], selp[:])

            # xt1 tiles REUSE the xt0 buffers (same tag, bufs=1): their DMAs
            # get a real WAR semaphore on xt0's last reader (the q0
            # projections), which keeps these 4MB of loads out of the
            # startup-critical DMA window without relying on scheduler
            # placement.
            xt1 = [
                xtpool.tile([128, dt8, chunk], BF16, tag=f"xt0c{ch}",
                            name=f"xt1c{ch}")
                for ch in range(nch)
            ]

            biasc = cpool.tile([128, 1], F32, tag="biasc")
            nc.vector.memset(biasc[:], EXP_BIAS)

            wo_sb = cpool.tile([128, dt8, D], BF16, tag="wo")

            # ---------- pipeline pieces ----------
            def emit_proj(wsb, dst, ch, xt_tiles, rope, rot_ps=None):
                cols = slice(ch * chunk, (ch + 1) * chunk)
                xt_sb = xt_tiles[ch]
                ps = psC.tile([128, chunk], F32, tag="tp", name="proj_ps")
                for dt in range(dt8):
                    nc.tensor.matmul(
                        ps[:],
                        wsb[:, dt, :],
                        xt_sb[:, dt, :],
                        start=(dt == 0),
                        stop=(dt == dt8 - 1),
                    )
                if not rope:
                    nc.vector.tensor_copy(dst[:, cols], ps[:])
                    return
                tsb = ropepool.tile([128, chunk], BF16, tag="tsb")
                nc.scalar.copy(tsb[:], ps[:])
                # rot output: batch-0 kv runs before attention exists, so it
                # can borrow an (idle) psA score bank and relieve the psC ring
                rpool, rtag = rot_ps or (psC, "tp")
                rps = rpool.tile([128, chunk], F32, tag=rtag)
                nc.tensor.matmul(rps[:], mp_sb[:], tsb[:], start=True, stop=True)
                m1 = ropepool.tile([128, chunk], BF16, tag="m1")
                nc.vector.tensor_tensor(
                    m1[:], tsb[:], cs128[:, cols], mybir.AluOpType.mult
                )
                m2 = ropepool.tile([128, chunk], BF16, tag="m2")
                nc.vector.tensor_tensor(
                    m2[:], rps[:], sn128[:, cols], mybir.AluOpType.mult
                )
                nc.vector.tensor_tensor(
                    dst[:, cols], m1[:], m2[:], mybir.AluOpType.add
                )

            def emit_vt_group(ch, vt_sb, v_sb, vt_ps=None):
                vpool, vtag = vt_ps or (psC, "tp")
                vps = vpool.tile([128, 4, 128], BF16, tag=vtag)
                for j in range(4):
                    ktt = ch * 4 + j
                    nc.tensor.transpose(
                        vps[:, j, :],
                        vt_sb[:, ktt * 128 : (ktt + 1) * 128],
                        id_sb[:],
                    )
                nc.vector.tensor_copy(
                    v_sb[:, ch * 4 : (ch + 1) * 4, :, 0:HD],
                    vps[:].rearrange("p t (h d) -> p t h d", h=HL),
                )

            def emit_attn_chunk(bi, ch, q_rope, k_rope, v_sb, aohs):
                cols = slice(ch * chunk, (ch + 1) * chunk)
                pts = {}
                for gi, (k0, glen) in enumerate(groups):
                    for h in range(HL):
                        rows = slice(h * HD, (h + 1) * HD)
                        sg = psA.tile([128, GMAX, chunk], F32, tag=f"sc{h}")
                        for j in range(glen):
                            ktt = k0 + j
                            nc.tensor.matmul(
                                sg[:, j, :],
                                k_rope[rows, ktt * 128 : (ktt + 1) * 128],
                                q_rope[rows, cols],
                                start=True,
                                stop=True,
                            )
                        pt = ptpool.tile([128, GMAX, chunk], BF16, tag="pt")
                        nc.scalar.activation(
                            pt[:, :glen, :],
                            sg[:, :glen, :],
                            mybir.ActivationFunctionType.Exp,
                            bias=biasc[:],
                            scale=EXP_SCALE,
                        )
                        pts[(gi, h)] = pt
                for h in range(HL):
                    pv = psB.tile([HD + 1, chunk], F32, tag="pv")
                    for gi, (k0, glen) in enumerate(groups):
                        pt = pts[(gi, h)]
                        for j in range(glen):
                            ktt = k0 + j
                            nc.tensor.matmul(
                                pv[:],
                                v_sb[:, ktt, h, :],
                                pt[:, j, :],
                                start=(ktt == 0),
                                stop=(ktt == kt - 1),
                            )
                    # unnormalized numerator + sigma row; 1/sigma applied
                    # once, consumer-side after the A2A
                    nc.vector.tensor_copy(aohs[h][:, cols], pv[:])

            def stage_a2a(a2a_in, aohs, ch, col0, w):
                """Stage chunk ch's columns of aohs into the A2A input
                buffer (peer j owns tokens [col0 + j*w, col0 + (j+1)*w)).
                rows 0..127: attn dims (h0, h1); rows 128..129: sigma."""
                lo = ch * chunk
                j0 = (lo - col0) // w
                nj = chunk // w
                for h in range(HL):
                    nc.sync.dma_start(
                        a2a_in[j0 : j0 + nj, h * HD : (h + 1) * HD, :].rearrange(
                            "j r c -> r j c"
                        ),
                        aohs[h][0:HD, lo : lo + chunk].rearrange(
                            "r (j c) -> r j c", j=nj
                        ),
                    )
                    nc.sync.dma_start(
                        a2a_in[j0 : j0 + nj, DL + h : DL + h + 1, :].rearrange(
                            "j r c -> r j c"
                        ),
                        aohs[h][HD : HD + 1, lo : lo + chunk].rearrange(
                            "r (j c) -> r j c", j=nj
                        ),
                    )

            def a2a_bufs(tag, w):
                a2a_in = dram.tile(
                    [N_CORES, DL + HL, w], BF16, tag=f"a2a_in{tag}",
                    name=f"a2a_in{tag}",
                )
                a2a_out = dram.tile(
                    [N_CORES, DL + HL, w], BF16, tag=f"a2a_out{tag}",
                    name=f"a2a_out{tag}",
                )
                return a2a_in, a2a_out

            def emit_collective(a2a_in, a2a_out):
                nc.gpsimd.collective_compute(
                    "AllToAll",
                    mybir.AluOpType.bypass,
                    replica_groups=[list(range(N_CORES))],
                    ins=[a2a_in.opt()],
                    outs=[a2a_out.opt()],
                )

            def emit_recv_dma(a2a_out, w, tg):
                """Pull A2A results on the gpsimd queue (which also owns the
                collective triggers, so data deps order naturally and any
                wait stalls an otherwise-idle engine)."""
                recv = rcvpool.tile(
                    [DL, N_CORES, w], BF16, tag=f"recv{tg}", name=f"recv{tg}"
                )
                nc.gpsimd.dma_start(
                    recv[:], a2a_out[:, 0:DL, :].rearrange("j r c -> r j c")
                )
                # sigr row h*8+i = sigma of source core i's local head h
                sigr = rcvpool.tile([H, w], BF16, tag=f"sigr{tg}", name=f"sigr{tg}")
                for h in range(HL):
                    nc.gpsimd.dma_start(
                        sigr[h * N_CORES : (h + 1) * N_CORES, :],
                        a2a_out[:, DL + h, :],
                    )
                return recv, sigr

            def emit_norm(recv, sigr, w, tg):
                """Consumer-side 1/sigma: reciprocal + selector-matmul
                broadcast + one multiply into recv."""
                sigf = nrmpool.tile([H, w], F32, tag=f"sigf{tg}", name=f"sigf{tg}")
                nc.vector.tensor_copy(sigf[:], sigr[:])
                rcpf = nrmpool.tile([H, w], F32, tag=f"rcpf{tg}", name=f"rcpf{tg}")
                nc.vector.reciprocal(rcpf[:], sigf[:])
                rcpb = nrmpool.tile([H, w], BF16, tag=f"rcpb{tg}", name=f"rcpb{tg}")
                nc.vector.tensor_copy(rcpb[:], rcpf[:])
                bcs = rcvpool.tile(
                    [DL, N_CORES, w], BF16, tag=f"bcs{tg}", name=f"bcs{tg}"
                )
                for i2 in range(N_CORES // 2):
                    bcp = psC.tile([128, 2, w], F32, tag="tp", name=f"bcp{tg}")
                    for k in range(2):
                        i = 2 * i2 + k
                        nc.tensor.matmul(
                            bcp[:, k, :],
                            sel_sb[:, i, :],
                            rcpb[:],
                            start=True,
                            stop=True,
                        )
                    nc.vector.tensor_copy(bcs[:, 2 * i2 : 2 * i2 + 2, :], bcp[:])
                nc.vector.tensor_tensor(
                    recv[:], recv[:], bcs[:], mybir.AluOpType.mult
                )

            def emit_wo_mm(recv, w, out_row0, tg):
                ntt = (w + 127) // 128
                for j in range(ntt):
                    tw = min(128, w - j * 128)
                    osb = outpool.tile([128, D], F32, tag="osb", name=f"osb{tg}{j}")
                    for nco in range(D // chunk):
                        wps = psA.tile(
                            [128, chunk], F32, tag=f"sc{(j + nco) % 2}", name="wps"
                        )
                        for i in range(N_CORES):
                            nc.tensor.matmul(
                                wps[0:tw, :],
                                recv[:, i, j * 128 : j * 128 + tw],
                                wo_sb[:, i, nco * chunk : (nco + 1) * chunk],
                                start=(i == 0),
                                stop=(i == N_CORES - 1),
                            )
                        nc.scalar.copy(
                            osb[0:tw, nco * chunk : (nco + 1) * chunk], wps[0:tw, :]
                        )
                        nc.sync.dma_start(
                            out[
                                out_row0 + j * 128 : out_row0 + j * 128 + tw,
                                nco * chunk : (nco + 1) * chunk,
                            ],
                            osb[0:tw, nco * chunk : (nco + 1) * chunk],
                        )

            # ---------- batch-0 QKV ----------
            q0 = qkvpool.tile([DL, s], BF16, tag="q_rope", bufs=1)
            k0_ = qkvpool.tile([DL, s], BF16, tag="k_rope")
            vt0 = qkvpool.tile([DL, s], BF16, tag="vt", bufs=1)
            v0 = qkvpool.tile([128, kt, HL, HD + 1], BF16, tag="v_sb")
            nc.vector.memset(v0[:, :, :, HD : HD + 1], 1.0)
            for ch in range(nch):
                emit_proj(wk_sb, k0_, ch, xt0, rope=True, rot_ps=(psA, "sc1"))
                emit_proj(wv_sb, vt0, ch, xt0, rope=False)
                emit_vt_group(ch, vt0, v0, vt_ps=(psA, "sc0"))

            for ch in range(nch):
                emit_proj(wq_sb, q0, ch, xt0, rope=True)
                # WAR-gated xt1 load for this chunk: waits (in the sync
                # ring) until the q0 projection has consumed xt0[ch]
                nc.sync.dma_start(xt1[ch][:], xt_dram(1, ch))

            # Wo load: queued on the sync ring behind the gated xt1 loads,
            # so it can't compete with the startup-critical DMAs; needed
            # only for the Wo matmuls at the tail.
            nc.sync.dma_start(wo_sb[:], wop[:])

            # ---------- batch-0 attention, batch-1 kv interleaved ----------
            ao0 = [
                attpool.tile([HD + 1, s], BF16, tag=f"aoh{h}", name=f"ao0_{h}")
                for h in range(HL)
            ]
            q1 = qkvpool.tile([DL, s], BF16, tag="q_rope", bufs=1)
            k1 = qkvpool.tile([DL, s], BF16, tag="k_rope")
            vt1 = qkvpool.tile([DL, s], BF16, tag="vt", bufs=1)
            v1 = qkvpool.tile([128, kt, HL, HD + 1], BF16, tag="v_sb")
            in0, out0 = a2a_bufs("b0", shard_b0)
            for ch in range(nch):
                emit_attn_chunk(0, ch, q0, k0_, v0, ao0)
                stage_a2a(in0, ao0, ch, 0, shard_b0)
                if ch == 0:
                    nc.vector.memset(v1[:, :, :, HD : HD + 1], 1.0)
                emit_proj(wk_sb, k1, ch, xt1, rope=True)
                emit_proj(wv_sb, vt1, ch, xt1, rope=False)
                emit_vt_group(ch, vt1, v1)
            emit_collective(in0, out0)

            # ---------- batch-1 q + attention ----------
            ao1 = [
                attpool.tile([HD + 1, s], BF16, tag=f"aoh{h}", name=f"ao1_{h}")
                for h in range(HL)
            ]
            in1a, out1a = a2a_bufs("b1a", shard_b1)
            in1b, out1b = a2a_bufs("b1b", shard_b1 // 2)
            in1c, out1c = a2a_bufs("b1c", shard_b1 // 2)
            emit_proj(wq_sb, q1, 0, xt1, rope=True)
            emit_proj(wq_sb, q1, 1, xt1, rope=True)
            emit_attn_chunk(1, 0, q1, k1, v1, ao1)
            stage_a2a(in1a, ao1, 0, 0, shard_b1)
            emit_proj(wq_sb, q1, 2, xt1, rope=True)
            emit_attn_chunk(1, 1, q1, k1, v1, ao1)
            stage_a2a(in1a, ao1, 1, 0, shard_b1)
            emit_proj(wq_sb, q1, 3, xt1, rope=True)

            # first half of batch 1 (cols 0..1024; peer j owns 128 tokens)
            emit_collective(in1a, out1a)
            # b0's A2A finished long ago: prefetch its results now on the
            # idle gpsimd queue, overlapping batch-1 attention.
            recv0, sigr0 = emit_recv_dma(out0, shard_b0, "b0")

            # quarters 3 and 4 of batch 1 ship separately (w=64/peer): the
            # chunk-2 piece's collective fires a whole chunk earlier, so only
            # the small chunk-3 piece's latency is exposed at the very end
            emit_attn_chunk(1, 2, q1, k1, v1, ao1)
            stage_a2a(in1b, ao1, 2, s // 2, shard_b1 // 2)
            emit_collective(in1b, out1b)
            recv1a, sigr1a = emit_recv_dma(out1a, shard_b1, "b1a")
            emit_attn_chunk(1, 3, q1, k1, v1, ao1)
            stage_a2a(in1c, ao1, 3, 3 * s // 4, shard_b1 // 2)
            emit_collective(in1c, out1c)
            recv1b, sigr1b = emit_recv_dma(out1b, shard_b1 // 2, "b1b")
            recv1c, sigr1c = emit_recv_dma(out1c, shard_b1 // 2, "b1c")

            with tc.tile_wait_until(1.0):
                emit_norm(recv0, sigr0, shard_b0, "b0")
                emit_wo_mm(recv0, shard_b0, 0, "b0")
            with tc.tile_wait_until(1.01):
                emit_norm(recv1a, sigr1a, shard_b1, "b1a")
                emit_wo_mm(recv1a, shard_b1, 2 * shard_b1, "b1a")
            with tc.tile_wait_until(1.02):
                emit_norm(recv1b, sigr1b, shard_b1 // 2, "b1b")
                emit_wo_mm(recv1b, shard_b1 // 2, 3 * shard_b1, "b1b")
            with tc.tile_wait_until(1.03):
                emit_norm(recv1c, sigr1c, shard_b1 // 2, "b1c")
                emit_wo_mm(recv1c, shard_b1 // 2, 3 * shard_b1 + shard_b1 // 2,
                           "b1c")

    split_excess_waits(nc)
    return nc


def _host_prep(x, cos, sin, b, s):
    """Device-ready layouts shared across cores."""
    nch = s // 512
    # x^T chunked: [128, b, nch, dt8, 512] with each (b, ch) block contiguous
    xt = np.ascontiguousarray(x.reshape(b * s, D).T.astype(BF16_NP))  # [D, b*s]
    xt = (
        xt.reshape(D // 128, 128, b, nch, 512)
        .transpose(1, 2, 3, 0, 4)
        .reshape(128, b * nch * (D // 128), 512)
    )
    xt = np.ascontiguousarray(xt)
    # transposed rope tables [32, s]; replicated to 128 rows on-device
    csn = np.ascontiguousarray(cos.T.astype(BF16_NP))
    snn = np.ascontiguousarray(sin.T.astype(BF16_NP))
    # selector for the consumer-side 1/sigma broadcast (sigr is h-major)
    selm = np.zeros((H, N_CORES, 128), dtype=np.float32)
    for i in range(N_CORES):
        for p in range(128):
            selm[(p // HD) * N_CORES + i, i, p] = 1.0
    selb = np.ascontiguousarray(selm.astype(BF16_NP))
    mperm = np.ascontiguousarray(_perm_matrix().astype(BF16_NP))
    ident = np.ascontiguousarray(np.eye(128, dtype=np.float32).astype(BF16_NP))
    return xt, csn, snn, selb, mperm, ident


def _swz(w):  # [D, M] -> [128, dt8, M] bf16
    m = w.shape[1]
    return np.ascontiguousarray(
        np.asarray(w, dtype=np.float32)
        .reshape(D // 128, 128, m)
        .transpose(1, 0, 2)
        .astype(BF16_NP)
    )


def make_in_maps(x, cos, sin, Wq, Wk, Wv, Wo, b, s):
    xt, csn, snn, selb, mperm, ident = _host_prep(
        np.asarray(x, dtype=np.float32),
        np.asarray(cos, dtype=np.float32),
        np.asarray(sin, dtype=np.float32),
        b, s,
    )
    wo_s = _swz(Wo)
    in_maps = []
    for c in range(N_CORES):
        cs = slice(c * DL, (c + 1) * DL)
        in_maps.append(
            {
                "xt": xt,
                "csn": csn,
                "snn": snn,
                "wq": _swz(Wq[:, cs]),
                "wk": _swz(Wk[:, cs]),
                "wv": _swz(Wv[:, cs]),
                "wo": wo_s,
                "sel": selb,
                "mperm": mperm,
                "ident": ident,
            }
        )
    return in_maps


_NC_CACHE = {}


def run(x, cos, sin, Wq, Wk, Wv, Wo, trace=False, chunk=512, pt_bufs=10):
    b, s, _ = x.shape
    key = (b, s, chunk, pt_bufs)
    if key not in _NC_CACHE:
        _NC_CACHE[key] = build_nc(b=b, s=s, chunk=chunk, pt_bufs=pt_bufs)
    nc = _NC_CACHE[key]
    in_maps = make_in_maps(x, cos, sin, Wq, Wk, Wv, Wo, b, s)
    res = run_bass_kernel_spmd(nc, in_maps, list(range(N_CORES)), trace=trace)
    sb0 = s // N_CORES  # 256
    sb1 = s // (2 * N_CORES)  # 128
    sb2 = sb1 // 2  # 64
    b0 = np.concatenate(
        [res.results[c]["out"][0:sb0] for c in range(N_CORES)], axis=0
    )
    b1 = np.concatenate(
        [res.results[c]["out"][sb0 : sb0 + sb1] for c in range(N_CORES)]
        + [res.results[c]["out"][sb0 + sb1 : sb0 + sb1 + sb2]
           for c in range(N_CORES)]
        + [res.results[c]["out"][sb0 + sb1 + sb2 : sb0 + 2 * sb1]
           for c in range(N_CORES)],
        axis=0,
    )
    full = np.stack([b0, b1], axis=0)
    return full.reshape(b, s, D), res


def kernel(x, cos, sin, Wq, Wk, Wv, Wo):
    out, _ = run(
        np.asarray(x), np.asarray(cos), np.asarray(sin),
        np.asarray(Wq), np.asarray(Wk), np.asarray(Wv), np.asarray(Wo),
    )
    return out.astype(np.float32)


# revision 53
# speedup vs baseline: 1.0251x; 1.0251x over previous
"""Multi-head attention with RoPE on 8 Trainium2 NeuronCores (v4).

Problem: x[2,2048,1024] -> MHA(16 heads, hd=64, NeoX RoPE, non-causal) -> out.

Sharding: tensor-parallel over heads. Each core owns 2 heads. All input
layout work (x^T, bf16 casts, rope tables, weight swizzles, per-core
weight column slices) happens host-side in make_in_maps; the device
kernel is pure compute:

  - chunked x^T loads ordered by consumption, with the startup-critical
    pieces on the gpsimd DMA ring (earliest to post) and batch-1's x^T
    reusing batch-0's buffers so its loads carry a WAR semaphore that
    keeps them out of the startup window,
  - rope cos/sin tables load as their unique 32 rows and replicate to 128
    partitions with two SBUF->SBUF copies,
  - q^T,k^T (RoPE'd via a permutation matmul) and v^T projections from the
    pre-transposed x^T, full sequence per core,
  - flash-style attention with *transposed* scores [s_k, s_q]; the softmax
    denominator comes from a fused ones-column in V (constant bias inside
    the exp keeps fp32 range safe),
  - AllToAll staging DMAs are issued per attention chunk, so when the last
    chunk finishes only its own slice remains to stage. Batch 0 goes as
    one piece (peer j owns tokens [j*256,(j+1)*256)); batch 1 ships in
    three pieces (half + quarter + quarter, peer j owning a contiguous
    block of each) so each piece's collective fires as soon as its chunks
    finish and only the last quarter-size piece is exposed at the end,
  - recv DMAs live on the gpsimd queue (which also owns the collective
    triggers, so data deps order naturally and waits stall an idle
    engine); the consumer-side 1/sigma normalize chains + Wo matmuls sit
    in wait-gated tail blocks, filling the last collective's latency,
  - per-512-column output DMAs drain as each Wo column block finishes.

All matmuls run in bf16 (fp32 PSUM accumulation); rel-err tolerance 2e-2.
"""

import sys

sys.path.insert(0, "/opt/trn_rl_repo")

import numpy as np  # noqa: E402
import ml_dtypes  # noqa: E402

import concourse.bass as bass  # noqa: E402
import concourse.mybir as mybir  # noqa: E402
import concourse.tile as tile  # noqa: E402
from concourse.bass_utils import run_bass_kernel_spmd  # noqa: E402


N_CORES = 8
D = 1024
H = 16
HD = 64
HL = H // N_CORES  # local heads per core
DL = HL * HD  # 128 local attn dims
EXP_SCALE = 0.125  # 1/sqrt(hd)
EXP_BIAS = -24.0  # exp(s/8 - 24): cancels in softmax, keeps fp32 range safe
GMAX = 2  # score-psum kt-tiles per exp instruction

F32 = mybir.dt.float32
BF16 = mybir.dt.bfloat16
BF16_NP = ml_dtypes.bfloat16


def _kt_groups(kt):
    groups = []
    k0 = 0
    while k0 < kt:
        g = min(GMAX, kt - k0)
        if (kt - k0) % GMAX == 1 and GMAX > 1:
            g = min(GMAX - 1, kt - k0)
        groups.append((k0, g))
        k0 += g
    return groups


def _perm_matrix():
    """lhsT for the rotate_half matmul: qrot^T = lhsT.T @ q^T."""
    mt = np.zeros((DL, DL), dtype=np.float32)
    for o in (0, HD):
        for r in range(HD // 2):
            mt[o + r, o + r + HD // 2] = -1.0
            mt[o + r + HD // 2, o + r] = 1.0
    return np.ascontiguousarray(mt.T)


def split_excess_waits(nc, max_waits=1):
    """This container's walrus rejects >1 semaphore wait per instruction;
    split excess waits onto NoOp carriers on the same engine."""
    for bb in nc.m.functions[0].blocks:
        insts = bb.instructions
        idx = 0
        while idx < len(insts):
            ins = insts[idx]
            si = ins.sync_info
            if si is not None and si.on_wait and len(si.on_wait) > max_waits:
                ow = list(si.on_wait)
                si.on_wait = ow[-max_waits:]
                extra = ow[:-max_waits]
                k = 0
                while extra:
                    chunk, extra = extra[:max_waits], extra[max_waits:]
                    c = mybir.InstNoOp(name=f"{ins.name}-ws{k}", ins=[], outs=[])
                    c.engine = ins.engine
                    c.sync_info = mybir.SyncInfo(on_wait=chunk, on_update=[])
                    nc.register_instruction(c)
                    insts.insert(idx, c)
                    idx += 1
                    k += 1
            idx += 1


def build_nc(b=2, s=2048, chunk=512, pt_bufs=10):
    kt = s // 128
    nch = s // chunk
    dt8 = D // 128
    shard_b0 = s // N_CORES  # 256 tokens per core, batch 0
    shard_b1 = s // (2 * N_CORES)  # 128 tokens per core per piece, batch 1
    groups = _kt_groups(kt)

    nc = bass.Bass()
    # all layout prep is host-side; everything below is bf16 device-ready
    # x^T chunked: [128, b*nch*dt8, chunk], (batch, chunk) blocks contiguous
    xtp = nc.declare_dram_parameter(
        "xt", [128, b * nch * dt8, chunk], BF16, isOutput=False
    )
    csp = nc.declare_dram_parameter("csn", [32, s], BF16, isOutput=False)
    snp = nc.declare_dram_parameter("snn", [32, s], BF16, isOutput=False)
    wqp = nc.declare_dram_parameter("wq", [128, dt8, DL], BF16, isOutput=False)
    wkp = nc.declare_dram_parameter("wk", [128, dt8, DL], BF16, isOutput=False)
    wvp = nc.declare_dram_parameter("wv", [128, dt8, DL], BF16, isOutput=False)
    wop = nc.declare_dram_parameter("wo", [128, dt8, D], BF16, isOutput=False)
    selp = nc.declare_dram_parameter("sel", [H, N_CORES, 128], BF16, isOutput=False)
    mpp = nc.declare_dram_parameter("mperm", [DL, DL], BF16, isOutput=False)
    idp = nc.declare_dram_parameter("ident", [128, 128], BF16, isOutput=False)
    out = nc.declare_dram_parameter("out", [4 * shard_b1, D], F32, isOutput=True)

    def xt_dram(bi, ch):
        o = (bi * nch + ch) * dt8
        return xtp[:, o : o + dt8, :]

    with tile.TileContext(nc) as tc:
        with (
            tc.tile_pool(name="dram", bufs=1, space="DRAM") as dram,
            tc.tile_pool(name="const", bufs=1) as cpool,
            tc.tile_pool(name="xt", bufs=1) as xtpool,
            tc.tile_pool(name="qkv", bufs=2) as qkvpool,
            tc.tile_pool(name="rope", bufs=2) as ropepool,
            tc.tile_pool(name="pt", bufs=pt_bufs) as ptpool,
            tc.tile_pool(name="att", bufs=2) as attpool,
            tc.tile_pool(name="nrm", bufs=2) as nrmpool,
            tc.tile_pool(name="recv", bufs=1) as rcvpool,
            tc.tile_pool(name="outp", bufs=2) as outpool,
            # PSUM: 8 banks. psA = scores (2 tags x 2 banks; Wo borrows).
            # psB = 2 PV banks. psC = 2 banks for proj / v-transposes / rot /
            # sigma broadcasts.
            tc.tile_pool(name="psA", bufs=1, space="PSUM") as psA,
            tc.tile_pool(name="psB", bufs=2, space="PSUM") as psB,
            tc.tile_pool(name="psC", bufs=2, space="PSUM") as psC,
        ):
            # ---------- loads, critical-path first ----------
            # gpsimd posts DMAs earliest (its preamble is shortest), so the
            # startup-critical loads go there, in consumption order. xt1/wo
            # post later from the sync ring behind a WAR semaphore (below)
            # so their descriptors can't jump ahead of these in the queues.
            wk_sb = cpool.tile([128, dt8, DL], BF16, tag="wk")
            nc.gpsimd.dma_start(wk_sb[:], wkp[:])
            xt0 = [
                xtpool.tile([128, dt8, chunk], BF16, tag=f"xt0c{ch}",
                            name=f"xt0c{ch}")
                for ch in range(nch)
            ]
            nc.gpsimd.dma_start(xt0[0][:], xt_dram(0, 0))
            # rope tables: load the unique 32 rows, replicate to 128
            # partitions with two SBUF->SBUF copies each (saves 0.75MB of
            # contended startup HBM traffic)
            cs128 = cpool.tile([128, s], BF16, tag="cs")
            nc.gpsimd.dma_start(cs128[0:32, :], csp[:])
            sn128 = cpool.tile([128, s], BF16, tag="sn")
            nc.gpsimd.dma_start(sn128[0:32, :], snp[:])
            nc.gpsimd.dma_start(xt0[1][:], xt_dram(0, 1))
            mp_sb = cpool.tile([DL, DL], BF16, tag="mperm")
            nc.gpsimd.dma_start(mp_sb[:], mpp[:])
            wv_sb = cpool.tile([128, dt8, DL], BF16, tag="wv")
            nc.gpsimd.dma_start(wv_sb[:], wvp[:])
            # table broadcasts ride the same gpsimd ring AFTER the critical
            # loads (FIFO): they can't steal queue time from xt0c0
            for tbl in (cs128, sn128):
                nc.gpsimd.dma_start(tbl[32:64, :], tbl[0:32, :])
                nc.gpsimd.dma_start(tbl[64:128, :], tbl[0:64, :])
            id_sb = cpool.tile([128, 128], BF16, tag="ident")
            nc.gpsimd.dma_start(id_sb[:], idp[:])
            nc.gpsimd.dma_start(xt0[2][:], xt_dram(0, 2))
            nc.gpsimd.dma_start(xt0[3][:], xt_dram(0, 3))
            wq_sb = cpool.tile([128, dt8, DL], BF16, tag="wq")
            nc.gpsimd.dma_start(wq_sb[:], wqp[:])
            sel_sb = cpool.tile([H, N_CORES, 128], BF16, tag="sel")
            nc.gpsimd.dma_start(sel_sb[:], selp[:])

            # xt1 tiles REUSE the xt0 buffers (same tag, bufs=1): their DMAs
            # get a real WAR semaphore on xt0's last reader (the q0
            # projections), which keeps these 4MB of loads out of the
            # startup-critical DMA window without relying on scheduler
            # placement.
            xt1 = [
                xtpool.tile([128, dt8, chunk], BF16, tag=f"xt0c{ch}",
                            name=f"xt1c{ch}")
                for ch in range(nch)
            ]

            biasc = cpool.tile([128, 1], F32, tag="biasc")
            nc.vector.memset(biasc[:], EXP_BIAS)

            wo_sb = cpool.tile([128, dt8, D], BF16, tag="wo")

            # ---------- pipeline pieces ----------
            def emit_proj(wsb, dst, ch, xt_tiles, rope, rot_ps=None):
                cols = slice(ch * chunk, (ch + 1) * chunk)
                xt_sb = xt_tiles[ch]
                ps = psC.tile([128, chunk], F32, tag="tp", name="proj_ps")
                for dt in range(dt8):
                    nc.tensor.matmul(
                        ps[:],
                        wsb[:, dt, :],
                        xt_sb[:, dt, :],
                        start=(dt == 0),
                        stop=(dt == dt8 - 1),
                    )
                if not rope:
                    nc.vector.tensor_copy(dst[:, cols], ps[:])
                    return
                tsb = ropepool.tile([128, chunk], BF16, tag="tsb")
                nc.scalar.copy(tsb[:], ps[:])
                # rot output: batch-0 kv runs before attention exists, so it
                # can borrow an (idle) psA score bank and relieve the psC ring
                rpool, rtag = rot_ps or (psC, "tp")
                rps = rpool.tile([128, chunk], F32, tag=rtag)
                nc.tensor.matmul(rps[:], mp_sb[:], tsb[:], start=True, stop=True)
                m1 = ropepool.tile([128, chunk], BF16, tag="m1")
                nc.vector.tensor_tensor(
                    m1[:], tsb[:], cs128[:, cols], mybir.AluOpType.mult
                )
                m2 = ropepool.tile([128, chunk], BF16, tag="m2")
                nc.vector.tensor_tensor(
                    m2[:], rps[:], sn128[:, cols], mybir.AluOpType.mult
                )
                nc.vector.tensor_tensor(
                    dst[:, cols], m1[:], m2[:], mybir.AluOpType.add
                )

            def emit_vt_group(ch, vt_sb, v_sb, vt_ps=None):
                vpool, vtag = vt_ps or (psC, "tp")
                vps = vpool.tile([128, 4, 128], BF16, tag=vtag)
                for j in range(4):
                    ktt = ch * 4 + j
                    nc.tensor.transpose(
                        vps[:, j, :],
                        vt_sb[:, ktt * 128 : (ktt + 1) * 128],
                        id_sb[:],
                    )
                nc.vector.tensor_copy(
                    v_sb[:, ch * 4 : (ch + 1) * 4, :, 0:HD],
                    vps[:].rearrange("p t (h d) -> p t h d", h=HL),
                )

            def emit_attn_chunk(bi, ch, q_rope, k_rope, v_sb, aohs):
                cols = slice(ch * chunk, (ch + 1) * chunk)
                pts = {}
                for gi, (k0, glen) in enumerate(groups):
                    for h in range(HL):
                        rows = slice(h * HD, (h + 1) * HD)
                        sg = psA.tile([128, GMAX, chunk], F32, tag=f"sc{h}")
                        for j in range(glen):
                            ktt = k0 + j
                            nc.tensor.matmul(
                                sg[:, j, :],
                                k_rope[rows, ktt * 128 : (ktt + 1) * 128],
                                q_rope[rows, cols],
                                start=True,
                                stop=True,
                            )
                        pt = ptpool.tile([128, GMAX, chunk], BF16, tag="pt")
                        nc.scalar.activation(
                            pt[:, :glen, :],
                            sg[:, :glen, :],
                            mybir.ActivationFunctionType.Exp,
                            bias=biasc[:],
                            scale=EXP_SCALE,
                        )
                        pts[(gi, h)] = pt
                for h in range(HL):
                    pv = psB.tile([HD + 1, chunk], F32, tag="pv")
                    for gi, (k0, glen) in enumerate(groups):
                        pt = pts[(gi, h)]
                        for j in range(glen):
                            ktt = k0 + j
                            nc.tensor.matmul(
                                pv[:],
                                v_sb[:, ktt, h, :],
                                pt[:, j, :],
                                start=(ktt == 0),
                                stop=(ktt == kt - 1),
                            )
                    # unnormalized numerator + sigma row; 1/sigma applied
                    # once, consumer-side after the A2A
                    nc.vector.tensor_copy(aohs[h][:, cols], pv[:])

            def stage_a2a(a2a_in, aohs, ch, col0, w):
                """Stage chunk ch's columns of aohs into the A2A input
                buffer (peer j owns tokens [col0 + j*w, col0 + (j+1)*w)).
                rows 0..127: attn dims (h0, h1); rows 128..129: sigma."""
                lo = ch * chunk
                j0 = (lo - col0) // w
                nj = chunk // w
                for h in range(HL):
                    nc.sync.dma_start(
                        a2a_in[j0 : j0 + nj, h * HD : (h + 1) * HD, :].rearrange(
                            "j r c -> r j c"
                        ),
                        aohs[h][0:HD, lo : lo + chunk].rearrange(
                            "r (j c) -> r j c", j=nj
                        ),
                    )
                    nc.sync.dma_start(
                        a2a_in[j0 : j0 + nj, DL + h : DL + h + 1, :].rearrange(
                            "j r c -> r j c"
                        ),
                        aohs[h][HD : HD + 1, lo : lo + chunk].rearrange(
                            "r (j c) -> r j c", j=nj
                        ),
                    )

            def a2a_bufs(tag, w):
                a2a_in = dram.tile(
                    [N_CORES, DL + HL, w], BF16, tag=f"a2a_in{tag}",
                    name=f"a2a_in{tag}",
                )
                a2a_out = dram.tile(
                    [N_CORES, DL + HL, w], BF16, tag=f"a2a_out{tag}",
                    name=f"a2a_out{tag}",
                )
                return a2a_in, a2a_out

            def emit_collective(a2a_in, a2a_out):
                nc.gpsimd.collective_compute(
                    "AllToAll",
                    mybir.AluOpType.bypass,
                    replica_groups=[list(range(N_CORES))],
                    ins=[a2a_in.opt()],
                    outs=[a2a_out.opt()],
                )

            def emit_recv_dma(a2a_out, w, tg):
                """Pull A2A results on the gpsimd queue (which also owns the
                collective triggers, so data deps order naturally and any
                wait stalls an otherwise-idle engine)."""
                recv = rcvpool.tile(
                    [DL, N_CORES, w], BF16, tag=f"recv{tg}", name=f"recv{tg}"
                )
                nc.gpsimd.dma_start(
                    recv[:], a2a_out[:, 0:DL, :].rearrange("j r c -> r j c")
                )
                # sigr row h*8+i = sigma of source core i's local head h
                sigr = rcvpool.tile([H, w], BF16, tag=f"sigr{tg}", name=f"sigr{tg}")
                for h in range(HL):
                    nc.gpsimd.dma_start(
                        sigr[h * N_CORES : (h + 1) * N_CORES, :],
                        a2a_out[:, DL + h, :],
                    )
                return recv, sigr

            def emit_rcp(sigr, w, tg):
                """Vector-only 1/sigma mini-chain ([16, w] ops)."""
                sigf = nrmpool.tile([H, w], F32, tag=f"sigf{tg}", name=f"sigf{tg}")
                nc.vector.tensor_copy(sigf[:], sigr[:])
                rcpf = nrmpool.tile([H, w], F32, tag=f"rcpf{tg}", name=f"rcpf{tg}")
                nc.vector.reciprocal(rcpf[:], sigf[:])
                rcpb = nrmpool.tile([H, w], BF16, tag=f"rcpb{tg}", name=f"rcpb{tg}")
                nc.vector.tensor_copy(rcpb[:], rcpf[:])
                return rcpb

            def emit_norm(recv, rcpb, w, tg):
                """Consumer-side 1/sigma apply: selector-matmul broadcast +
                one multiply into recv."""
                bcs = rcvpool.tile(
                    [DL, N_CORES, w], BF16, tag=f"bcs{tg}", name=f"bcs{tg}"
                )
                for i2 in range(N_CORES // 2):
                    bcp = psC.tile([128, 2, w], F32, tag="tp", name=f"bcp{tg}")
                    for k in range(2):
                        i = 2 * i2 + k
                        nc.tensor.matmul(
                            bcp[:, k, :],
                            sel_sb[:, i, :],
                            rcpb[:],
                            start=True,
                            stop=True,
                        )
                    nc.vector.tensor_copy(bcs[:, 2 * i2 : 2 * i2 + 2, :], bcp[:])
                nc.vector.tensor_tensor(
                    recv[:], recv[:], bcs[:], mybir.AluOpType.mult
                )

            def emit_wo_mm(recv, w, out_row0, tg):
                ntt = (w + 127) // 128
                for j in range(ntt):
                    tw = min(128, w - j * 128)
                    osb = outpool.tile([128, D], F32, tag="osb", name=f"osb{tg}{j}")
                    for nco in range(D // chunk):
                        wps = psA.tile(
                            [128, chunk], F32, tag=f"sc{(j + nco) % 2}", name="wps"
                        )
                        for i in range(N_CORES):
                            nc.tensor.matmul(
                                wps[0:tw, :],
                                recv[:, i, j * 128 : j * 128 + tw],
                                wo_sb[:, i, nco * chunk : (nco + 1) * chunk],
                                start=(i == 0),
                                stop=(i == N_CORES - 1),
                            )
                        nc.scalar.copy(
                            osb[0:tw, nco * chunk : (nco + 1) * chunk], wps[0:tw, :]
                        )
                        nc.sync.dma_start(
                            out[
                                out_row0 + j * 128 : out_row0 + j * 128 + tw,
                                nco * chunk : (nco + 1) * chunk,
                            ],
                            osb[0:tw, nco * chunk : (nco + 1) * chunk],
                        )

            # ---------- batch-0 QKV ----------
            q0 = qkvpool.tile([DL, s], BF16, tag="q_rope", bufs=1)
            k0_ = qkvpool.tile([DL, s], BF16, tag="k_rope")
            vt0 = qkvpool.tile([DL, s], BF16, tag="vt", bufs=1)
            v0 = qkvpool.tile([128, kt, HL, HD + 1], BF16, tag="v_sb")
            nc.vector.memset(v0[:, :, :, HD : HD + 1], 1.0)
            for ch in range(nch):
                emit_proj(wk_sb, k0_, ch, xt0, rope=True, rot_ps=(psA, "sc1"))
                emit_proj(wv_sb, vt0, ch, xt0, rope=False)
                emit_vt_group(ch, vt0, v0, vt_ps=(psA, "sc0"))

            for ch in range(nch):
                emit_proj(wq_sb, q0, ch, xt0, rope=True)
                # WAR-gated xt1 load for this chunk: waits (in the sync
                # ring) until the q0 projection has consumed xt0[ch]
                nc.sync.dma_start(xt1[ch][:], xt_dram(1, ch))

            # Wo load: queued on the sync ring behind the gated xt1 loads,
            # so it can't compete with the startup-critical DMAs; needed
            # only for the Wo matmuls at the tail.
            nc.sync.dma_start(wo_sb[:], wop[:])

            # ---------- batch-0 attention, batch-1 kv interleaved ----------
            ao0 = [
                attpool.tile([HD + 1, s], BF16, tag=f"aoh{h}", name=f"ao0_{h}")
                for h in range(HL)
            ]
            q1 = qkvpool.tile([DL, s], BF16, tag="q_rope", bufs=1)
            k1 = qkvpool.tile([DL, s], BF16, tag="k_rope")
            vt1 = qkvpool.tile([DL, s], BF16, tag="vt", bufs=1)
            v1 = qkvpool.tile([128, kt, HL, HD + 1], BF16, tag="v_sb")
            in0, out0 = a2a_bufs("b0", shard_b0)
            for ch in range(nch):
                emit_attn_chunk(0, ch, q0, k0_, v0, ao0)
                stage_a2a(in0, ao0, ch, 0, shard_b0)
                if ch == 0:
                    nc.vector.memset(v1[:, :, :, HD : HD + 1], 1.0)
                emit_proj(wk_sb, k1, ch, xt1, rope=True)
                emit_proj(wv_sb, vt1, ch, xt1, rope=False)
                emit_vt_group(ch, vt1, v1)
            emit_collective(in0, out0)

            # ---------- batch-1 q + attention ----------
            ao1 = [
                attpool.tile([HD + 1, s], BF16, tag=f"aoh{h}", name=f"ao1_{h}")
                for h in range(HL)
            ]
            in1a, out1a = a2a_bufs("b1a", shard_b1)
            in1b, out1b = a2a_bufs("b1b", shard_b1 // 2)
            in1c, out1c = a2a_bufs("b1c", shard_b1 // 2)
            emit_proj(wq_sb, q1, 0, xt1, rope=True)
            emit_proj(wq_sb, q1, 1, xt1, rope=True)
            emit_attn_chunk(1, 0, q1, k1, v1, ao1)
            stage_a2a(in1a, ao1, 0, 0, shard_b1)
            emit_proj(wq_sb, q1, 2, xt1, rope=True)
            emit_attn_chunk(1, 1, q1, k1, v1, ao1)
            stage_a2a(in1a, ao1, 1, 0, shard_b1)
            emit_proj(wq_sb, q1, 3, xt1, rope=True)

            # first half of batch 1 (cols 0..1024; peer j owns 128 tokens)
            emit_collective(in1a, out1a)
            # b0's A2A finished long ago: prefetch its results now on the
            # idle gpsimd queue, overlapping batch-1 attention.
            recv0, sigr0 = emit_recv_dma(out0, shard_b0, "b0")

            # quarters 3 and 4 of batch 1 ship separately (w=64/peer): the
            # chunk-2 piece's collective fires a whole chunk earlier, so only
            # the small chunk-3 piece's latency is exposed at the very end
            emit_attn_chunk(1, 2, q1, k1, v1, ao1)
            stage_a2a(in1b, ao1, 2, s // 2, shard_b1 // 2)
            # b0-only reciprocal prefetch: its collective finished ~40us ago
            # on every core, so this vector mini-chain runs stall-free here
            rcpb0 = emit_rcp(sigr0, shard_b0, "b0")
            emit_collective(in1b, out1b)
            recv1a, sigr1a = emit_recv_dma(out1a, shard_b1, "b1a")
            emit_attn_chunk(1, 3, q1, k1, v1, ao1)
            stage_a2a(in1c, ao1, 3, 3 * s // 4, shard_b1 // 2)
            emit_collective(in1c, out1c)
            recv1b, sigr1b = emit_recv_dma(out1b, shard_b1 // 2, "b1b")
            recv1c, sigr1c = emit_recv_dma(out1c, shard_b1 // 2, "b1c")

            with tc.tile_wait_until(1.0):
                emit_norm(recv0, rcpb0, shard_b0, "b0")
                emit_wo_mm(recv0, shard_b0, 0, "b0")
            with tc.tile_wait_until(1.01):
                rcpb1a = emit_rcp(sigr1a, shard_b1, "b1a")
                emit_norm(recv1a, rcpb1a, shard_b1, "b1a")
                emit_wo_mm(recv1a, shard_b1, 2 * shard_b1, "b1a")
            with tc.tile_wait_until(1.02):
                rcpb1b = emit_rcp(sigr1b, shard_b1 // 2, "b1b")
                emit_norm(recv1b, rcpb1b, shard_b1 // 2, "b1b")
                emit_wo_mm(recv1b, shard_b1 // 2, 3 * shard_b1, "b1b")
            with tc.tile_wait_until(1.03):
                rcpb1c = emit_rcp(sigr1c, shard_b1 // 2, "b1c")
                emit_norm(recv1c, rcpb1c, shard_b1 // 2, "b1c")
                emit_wo_mm(recv1c, shard_b1 // 2, 3 * shard_b1 + shard_b1 // 2,
                           "b1c")

    split_excess_waits(nc)
    return nc


def _host_prep(x, cos, sin, b, s):
    """Device-ready layouts shared across cores."""
    nch = s // 512
    # x^T chunked: [128, b, nch, dt8, 512] with each (b, ch) block contiguous
    xt = np.ascontiguousarray(x.reshape(b * s, D).T.astype(BF16_NP))  # [D, b*s]
    xt = (
        xt.reshape(D // 128, 128, b, nch, 512)
        .transpose(1, 2, 3, 0, 4)
        .reshape(128, b * nch * (D // 128), 512)
    )
    xt = np.ascontiguousarray(xt)
    # transposed rope tables [32, s]; replicated to 128 rows on-device
    csn = np.ascontiguousarray(cos.T.astype(BF16_NP))
    snn = np.ascontiguousarray(sin.T.astype(BF16_NP))
    # selector for the consumer-side 1/sigma broadcast (sigr is h-major)
    selm = np.zeros((H, N_CORES, 128), dtype=np.float32)
    for i in range(N_CORES):
        for p in range(128):
            selm[(p // HD) * N_CORES + i, i, p] = 1.0
    selb = np.ascontiguousarray(selm.astype(BF16_NP))
    mperm = np.ascontiguousarray(_perm_matrix().astype(BF16_NP))
    ident = np.ascontiguousarray(np.eye(128, dtype=np.float32).astype(BF16_NP))
    return xt, csn, snn, selb, mperm, ident


def _swz(w):  # [D, M] -> [128, dt8, M] bf16
    m = w.shape[1]
    return np.ascontiguousarray(
        np.asarray(w, dtype=np.float32)
        .reshape(D // 128, 128, m)
        .transpose(1, 0, 2)
        .astype(BF16_NP)
    )


def make_in_maps(x, cos, sin, Wq, Wk, Wv, Wo, b, s):
    xt, csn, snn, selb, mperm, ident = _host_prep(
        np.asarray(x, dtype=np.float32),
        np.asarray(cos, dtype=np.float32),
        np.asarray(sin, dtype=np.float32),
        b, s,
    )
    wo_s = _swz(Wo)
    in_maps = []
    for c in range(N_CORES):
        cs = slice(c * DL, (c + 1) * DL)
        in_maps.append(
            {
                "xt": xt,
                "csn": csn,
                "snn": snn,
                "wq": _swz(Wq[:, cs]),
                "wk": _swz(Wk[:, cs]),
                "wv": _swz(Wv[:, cs]),
                "wo": wo_s,
                "sel": selb,
                "mperm": mperm,
                "ident": ident,
            }
        )
    return in_maps


_NC_CACHE = {}


def run(x, cos, sin, Wq, Wk, Wv, Wo, trace=False, chunk=512, pt_bufs=10):
    b, s, _ = x.shape
    key = (b, s, chunk, pt_bufs)
    if key not in _NC_CACHE:
        _NC_CACHE[key] = build_nc(b=b, s=s, chunk=chunk, pt_bufs=pt_bufs)
    nc = _NC_CACHE[key]
    in_maps = make_in_maps(x, cos, sin, Wq, Wk, Wv, Wo, b, s)
    res = run_bass_kernel_spmd(nc, in_maps, list(range(N_CORES)), trace=trace)
    sb0 = s // N_CORES  # 256
    sb1 = s // (2 * N_CORES)  # 128
    sb2 = sb1 // 2  # 64
    b0 = np.concatenate(
        [res.results[c]["out"][0:sb0] for c in range(N_CORES)], axis=0
    )
    b1 = np.concatenate(
        [res.results[c]["out"][sb0 : sb0 + sb1] for c in range(N_CORES)]
        + [res.results[c]["out"][sb0 + sb1 : sb0 + sb1 + sb2]
           for c in range(N_CORES)]
        + [res.results[c]["out"][sb0 + sb1 + sb2 : sb0 + 2 * sb1]
           for c in range(N_CORES)],
        axis=0,
    )
    full = np.stack([b0, b1], axis=0)
    return full.reshape(b, s, D), res


def kernel(x, cos, sin, Wq, Wk, Wv, Wo):
    out, _ = run(
        np.asarray(x), np.asarray(cos), np.asarray(sin),
        np.asarray(Wq), np.asarray(Wk), np.asarray(Wv), np.asarray(Wo),
    )
    return out.astype(np.float32)
